# revision 49
# baseline (speedup 1.0000x reference)
"""Trainium2 Bass kernel for a dense multi-head attention layer.

Problem (hardcoded, self-contained):
  query [4, 2048, 1024] f32, key/value [4, 2048, 1024] f32,
  Wq/Wk/Wv/Wo [1024, 1024] f32, bq/bk/bv/bo [1024] f32.
  out = softmax((q Wq + bq)(k Wk + bk)^T / 8) (v Wv + bv) Wo + bo
  with 16 heads of dim 64.

Sharding: 8 cores = 4 batches x 2 query-T halves (pure data parallel, no
collectives). Each core computes a full [1024, 1024] output slice; the host
concatenates.

Per-core dataflow (bf16 matmuls, fp32 PSUM accumulation, fp32 softmax):
  - raw inputs cast f32->bf16 during SWDGE DMA into SBUF 128-token stages,
    then transposed to feature-major tiles on TensorE (identity matmul).
    All DMAs ride the SWDGE (gpsimd) queue: HWDGE-direct queue instructions
    only support a single semaphore wait, which this kernel's dependency
    structure exceeds.
  - Qt = Wq^T q^T, Kt = Wk^T k^T (feature-major); V = (v^T)^T Wv token-major
    into a ones-augmented [V_h | 1] layout. bq/bk fused into PSUM eviction.
  - attention per head-pair (two heads share a 128-partition chunk) per
    512-column t-chunk: scores St[s,t] = Kt_h^T Qt_h as K=64 row-paired
    matmuls (heads in row groups 0-63/64-127), softmax without
    max-subtraction (scores are O(6) by construction): P = exp(St/8) on
    ScalarE straight out of PSUM. PV: O'_h = [V_h|1]^T P accumulated over
    s-tiles; row 64 of O' is the softmax denominator l[t].
  - K-projection chunk j+1 is interleaved one matmul per attention step so
    the PE stays dense while ScalarE works on exp.
  - normalize O = O'[0:64]/l + bv: l is broadcast across partitions via a
    DRAM bounce, reciprocal_approx_fast in place, one TT multiply.
  - out = OT^T Wo + bo (bo pre-broadcast via DMA, fused in eviction), f32.
"""

import numpy as np

import concourse.bass as bass
import concourse.bacc as bacc
import concourse.mybir as mybir
import concourse.tile as tile

F32 = mybir.dt.float32
BF16 = mybir.dt.bfloat16
EXP = mybir.ActivationFunctionType.Exp

P = 128  # partitions
HD = 64  # head dim


class Cfg:
    def __init__(self, T, S, D, H, dma_xpose=True):
        self.dma_xpose = dma_xpose
        self.T = T  # query tokens per core
        self.S = S  # kv tokens
        self.D = D  # model dim
        self.H = H  # heads
        assert D == H * HD
        self.KC = D // P          # contraction chunks of 128
        self.PAIRS = H // 2       # head pairs
        self.ST = S // P          # s tiles of 128
        self.TC = min(512, T)     # matmul free-dim chunk over t
        self.NT = T // self.TC    # t chunks
        self.DC = min(512, D)     # matmul free-dim chunk over d
        self.ND = D // self.DC    # d chunks
        self.SC = min(512, S)     # matmul free-dim chunk over s
        self.NS = S // self.SC    # s chunks (for K proj)
        self.AW = 2 * self.TC     # scores/acc tile width (2 heads x t-chunk)


FULL = Cfg(T=1024, S=2048, D=1024, H=16, dma_xpose=False)
N_CORES = 8


def _pbcast(ap, n, drop_first=True):
    """Broadcast an AP across n partitions (step-0 partition dim)."""
    dims = [list(d) for d in list(ap.ap)]
    if drop_first:
        assert dims[0][1] == 1, dims
        dims = dims[1:]
    return bass.AP(tensor=ap.tensor, offset=ap.offset, ap=[[0, n]] + dims)


def build_kernel(ctx, tc, cfg, io):
    nc = tc.nc
    c = cfg
    scale = 1.0 / np.sqrt(HD)

    dram = ctx.enter_context(tc.tile_pool(name="dram", bufs=1, space="DRAM"))
    consts = ctx.enter_context(tc.tile_pool(name="consts", bufs=1))
    wpool = ctx.enter_context(tc.tile_pool(name="w", bufs=2))
    rawpool = ctx.enter_context(tc.tile_pool(name="raw", bufs=1))
    actpool = ctx.enter_context(tc.tile_pool(name="acts", bufs=1))
    psum = ctx.enter_context(tc.tile_pool(name="psum", bufs=1, space="PSUM"))
    ppool = ctx.enter_context(tc.tile_pool(name="p", bufs=2))
    npool = ctx.enter_context(tc.tile_pool(name="norm", bufs=2))
    n1pool = ctx.enter_context(tc.tile_pool(name="norm1", bufs=1))
    outpool = ctx.enter_context(tc.tile_pool(name="outsb", bufs=2))

    from concourse.masks import make_identity

    ident = consts.tile([P, P], BF16)
    make_identity(nc, ident)
    stpool = ctx.enter_context(tc.tile_pool(name="stage", bufs=3))

    def load_w(name):
        w = wpool.tile([P, c.KC, c.D], BF16, name=f"{name}_sb", tag="w")
        nc.gpsimd.dma_start(
            out=w[:], in_=io[name][:].rearrange("(c p) n -> p c n", p=P)
        )
        return w

    def load_biases():
        bq_col = consts.tile([P, c.KC], F32)
        nc.gpsimd.dma_start(
            out=bq_col[:], in_=io["bq"][:].rearrange("(c p) -> p c", p=P)
        )
        bk_col = consts.tile([P, c.KC], F32)
        nc.gpsimd.dma_start(
            out=bk_col[:], in_=io["bk"][:].rearrange("(c p) -> p c", p=P)
        )
        bv64 = consts.tile([HD, c.H], F32)
        nc.gpsimd.dma_start(
            out=bv64[:], in_=io["bv"][:].rearrange("(h p) -> p h", p=HD)
        )
        bo_bc = consts.tile([P, c.D], F32)
        nc.gpsimd.dma_start(
            out=bo_bc[:], in_=_pbcast(io["bo"][:], P, drop_first=False)
        )
        return bq_col, bk_col, bv64, bo_bc

    def transpose_in(dst, src_handle, ntok):
        """Feature-major transpose of a raw input.

        dma_xpose path: cast f32->bf16 into DRAM scratch (SWDGE), then
        DMA-xbar transpose straight into the destination tiles (bacc's
        generate_event_semaphores legalizes the one-wait-per-DMA constraint).
        Fallback path: cast 128-token blocks into SBUF and transpose on
        TensorE via identity matmuls."""
        if c.dma_xpose:
            x_bf = dram.tile([ntok, c.D], BF16, name=f"bf_{src_handle.name}")
            nc.gpsimd.dma_start(out=x_bf[:], in_=src_handle[:])
            hh = ntok // 2
            for half in range(2):
                rows = slice(half * hh, (half + 1) * hh)
                for kc in range(c.KC):
                    nc.sync.dma_start(
                        out=dst[:, kc, half * hh : (half + 1) * hh],
                        in_=x_bf[rows, kc * P : (kc + 1) * P],
                        transpose=True,
                    )
            return
        for tt in range(ntok // P):
            stage = stpool.tile([P, c.D], BF16, tag="stage")
            nc.gpsimd.dma_start(
                out=stage[:], in_=src_handle[tt * P : (tt + 1) * P, :]
            )
            for kc in range(c.KC):
                pst = psum.tile([P, P], BF16, name=f"tp_{tt}_{kc}", tag="sc",
                                bufs=2)
                nc.tensor.transpose(pst[:], stage[:, kc * P : (kc + 1) * P],
                                    ident[:])
                nc.vector.tensor_copy(
                    out=dst[:, kc, tt * P : (tt + 1) * P], in_=pst[:]
                )

    # PSUM tags: "sc" [128, AW] bufs=2 (4 banks), "acc" [65, AW] (2 banks),
    # "proj" [128, 512] bufs=2 (2 banks) -> exactly 8 banks.
    proj_i = [0]

    def proj_psum(width):
        t = psum.tile([P, width], F32, name=f"ps{proj_i[0]}", tag="proj", bufs=2)
        proj_i[0] += 1
        return t

    # ---- Q projection: Qt [d_out, t] feature-major ----
    # Casts and weight loads are ordered just-in-time per phase: the SWDGE
    # (gpsimd) descriptor generator is serial, so later phases' casts must
    # not delay this phase's weight load.
    bq_col, bk_col, bv64, bo_bc = load_biases()
    Wq_sb = load_w("Wq")
    qT = rawpool.tile([P, c.KC, c.T], BF16, tag="raw")
    transpose_in(qT, io["query"], c.T)
    Qt = actpool.tile([P, c.KC, c.T], BF16, tag="qt")
    for mc in range(c.KC):
        for n in range(c.NT):
            ps = proj_psum(c.TC)
            for kc in range(c.KC):
                mm = nc.tensor.matmul(
                    ps[:],
                    lhsT=Wq_sb[:, kc, mc * P : (mc + 1) * P],
                    rhs=qT[:, kc, n * c.TC : (n + 1) * c.TC],
                    start=(kc == 0),
                    stop=(kc == c.KC - 1),
                )
            nc.vector.tensor_scalar_add(
                out=Qt[:, mc, n * c.TC : (n + 1) * c.TC],
                in0=ps[:],
                scalar1=bq_col[:, mc : mc + 1],
            )

    # ---- V projection into ones-augmented token-major layout ----
    Wv_sb = load_w("Wv")
    valT = rawpool.tile([P, c.KC, c.S], BF16, tag="raw")
    transpose_in(valT, io["value"], c.S)
    vaug = actpool.tile([P, c.ST, c.H, 66], BF16, tag="vaug")
    nc.vector.memset(vaug[:, :, :, 64:65], 1.0)
    hpd = c.DC // HD  # heads per d-chunk
    for sc in range(c.ST):
        for n in range(c.ND):
            ps = proj_psum(c.DC)
            for kc in range(c.KC):
                mm = nc.tensor.matmul(
                    ps[:],
                    lhsT=valT[:, kc, sc * P : (sc + 1) * P],
                    rhs=Wv_sb[:, kc, n * c.DC : (n + 1) * c.DC],
                    start=(kc == 0),
                    stop=(kc == c.KC - 1),
                )
            nc.vector.tensor_copy(
                out=vaug[:, sc, n * hpd : (n + 1) * hpd, 0:64],
                in_=ps[:].rearrange("p (h x) -> p h x", x=HD),
            )

    # ---- K projection: chunk 0 up front, chunks 1.. interleaved into
    # attention (one matmul per attention step keeps PE dense while ACT
    # works on exp) ----
    Wk_sb = load_w("Wk")
    keyT = rawpool.tile([P, c.KC, c.S], BF16, tag="raw")
    transpose_in(keyT, io["key"], c.S)
    Kt = actpool.tile([P, c.KC, c.S], BF16, tag="kt")

    def kproj_chunk_ops(mc):
        """Yield thunks: the 32 matmuls + evictions for K chunk mc."""
        ops = []
        for n in range(c.NS):
            ps_holder = []
            for kc in range(c.KC):
                def mm(n=n, kc=kc, ps_holder=ps_holder):
                    if kc == 0:
                        ps_holder.append(proj_psum(c.SC))
                    nc.tensor.matmul(
                        ps_holder[-1][:],
                        lhsT=Wk_sb[:, kc, mc * P : (mc + 1) * P],
                        rhs=keyT[:, kc, n * c.SC : (n + 1) * c.SC],
                        start=(kc == 0),
                        stop=(kc == c.KC - 1),
                    )
                    if kc == c.KC - 1:
                        nc.vector.tensor_scalar_add(
                            out=Kt[:, mc, n * c.SC : (n + 1) * c.SC],
                            in0=ps_holder[-1][:],
                            scalar1=bk_col[:, mc : mc + 1],
                        )
                ops.append(mm)
        return ops

    for op in kproj_chunk_ops(0):
        op()

    Wo_sb = load_w("Wo")

    # ---- attention ----
    OT = actpool.tile([P, c.PAIRS, c.T], BF16, tag="ot")
    for j in range(c.PAIRS):
        pending = kproj_chunk_ops(j + 1) if j + 1 < c.KC else []
        pend_i = 0
        for n in range(c.NT):
            acc = psum.tile([65, c.AW], F32, name=f"acc_{j}_{n}", tag="acc")
            for st in range(c.ST):
                sp = psum.tile([P, c.AW], F32, name=f"sc_{j}_{n}_{st}", tag="sc",
                               bufs=2)
                for hh in range(2):
                    po = hh * HD
                    nc.tensor.matmul(
                        sp[:, hh * c.TC : (hh + 1) * c.TC],
                        lhsT=Kt[po : po + HD, j, st * P : (st + 1) * P],
                        rhs=Qt[po : po + HD, j, n * c.TC : (n + 1) * c.TC],
                        start=True,
                        stop=True,
                        tile_position=(po, 0),
                    )
                pt = ppool.tile([P, c.AW], BF16, tag="p")
                nc.scalar.activation(pt[:], sp[:], EXP, scale=float(scale))
                for hh in range(2):
                    sl = slice(hh * c.TC, (hh + 1) * c.TC)
                    nc.tensor.matmul(
                        acc[:, sl],
                        lhsT=vaug[:, st, 2 * j + hh, 0:65],
                        rhs=pt[:, sl],
                        start=(st == 0),
                        stop=(st == c.ST - 1),
                    )
                # interleave one K-proj matmul of the next chunk per step
                if pend_i < len(pending):
                    pending[pend_i]()
                    pend_i += 1
            # normalization: O = O'[0:64]/l + bv ; l = O'[64]
            nrm = npool.tile([65, c.AW], F32, tag="nrm")
            nc.vector.tensor_copy(out=nrm[:], in_=acc[:])
            l_dram = dram.tile([c.AW], F32, name=f"ld_{j}_{n}", tag="ld", bufs=2)
            nc.gpsimd.dma_start(out=l_dram[:], in_=nrm[64:65, :])
            rv = n1pool.tile([HD, c.AW], F32, name=f"rv_{j}_{n}", tag="rv")
            nc.gpsimd.dma_start(out=rv[:], in_=_pbcast(l_dram[:], HD, drop_first=False))
            nc.vector.reciprocal_approx_fast(out=rv[:], in_=rv[:])
            tmp = n1pool.tile([HD, c.AW], BF16, name=f"tmp_{j}_{n}", tag="tmp")
            nc.vector.tensor_mul(out=tmp[:], in0=nrm[0:64, :], in1=rv[:])
            tsl = slice(n * c.TC, (n + 1) * c.TC)
            nc.vector.tensor_scalar_add(
                out=OT[0:64, j, tsl], in0=tmp[:, 0 : c.TC],
                scalar1=bv64[:, 2 * j : 2 * j + 1],
            )
            shf = n1pool.tile([HD, c.TC], BF16, name=f"shf_{j}_{n}", tag="shf")
            nc.vector.tensor_scalar_add(
                out=shf[:], in0=tmp[:, c.TC : 2 * c.TC],
                scalar1=bv64[:, 2 * j + 1 : 2 * j + 2],
            )
            nc.gpsimd.dma_start(out=OT[64:128, j, tsl], in_=shf[:])
        while pend_i < len(pending):
            pending[pend_i]()
            pend_i += 1

    # ---- output projection ----
    for m in range(c.T // P):
        for n in range(c.ND):
            ps = proj_psum(c.DC)
            for j in range(c.PAIRS):
                nc.tensor.matmul(
                    ps[:],
                    lhsT=OT[:, j, m * P : (m + 1) * P],
                    rhs=Wo_sb[:, j, n * c.DC : (n + 1) * c.DC],
                    start=(j == 0),
                    stop=(j == c.PAIRS - 1),
                )
            osb = outpool.tile([P, c.DC], F32, tag="osb")
            nc.vector.tensor_add(
                out=osb[:], in0=ps[:], in1=bo_bc[:, n * c.DC : (n + 1) * c.DC]
            )
            nc.gpsimd.dma_start(
                out=io["out"][m * P : (m + 1) * P, n * c.DC : (n + 1) * c.DC],
                in_=osb[:],
            )


def build_nc(cfg=FULL):
    from contextlib import ExitStack

    nc = bacc.Bacc()
    io = {
        "query": nc.dram_tensor("query", [cfg.T, cfg.D], F32, kind="ExternalInput"),
        "key": nc.dram_tensor("key", [cfg.S, cfg.D], F32, kind="ExternalInput"),
        "value": nc.dram_tensor("value", [cfg.S, cfg.D], F32, kind="ExternalInput"),
        "Wq": nc.dram_tensor("Wq", [cfg.D, cfg.D], F32, kind="ExternalInput"),
        "Wk": nc.dram_tensor("Wk", [cfg.D, cfg.D], F32, kind="ExternalInput"),
        "Wv": nc.dram_tensor("Wv", [cfg.D, cfg.D], F32, kind="ExternalInput"),
        "Wo": nc.dram_tensor("Wo", [cfg.D, cfg.D], F32, kind="ExternalInput"),
        "bq": nc.dram_tensor("bq", [cfg.D], F32, kind="ExternalInput"),
        "bk": nc.dram_tensor("bk", [cfg.D], F32, kind="ExternalInput"),
        "bv": nc.dram_tensor("bv", [cfg.D], F32, kind="ExternalInput"),
        "bo": nc.dram_tensor("bo", [cfg.D], F32, kind="ExternalInput"),
        "out": nc.dram_tensor("out", [cfg.T, cfg.D], F32, kind="ExternalOutput"),
    }
    with tile.TileContext(nc) as tc:
        with ExitStack() as ctx:
            build_kernel(ctx, tc, cfg, io)
    nc.finalize()
    return nc


def run(inputs, trace=False):
    from concourse.bass_utils import run_bass_kernel_spmd

    arr = {k: np.ascontiguousarray(np.asarray(v, dtype=np.float32))
           for k, v in inputs.items()}
    B, T_full, D = arr["query"].shape
    half = T_full // 2
    nc = build_nc(FULL)
    in_maps = []
    for core in range(N_CORES):
        b, h = divmod(core, 2)
        m = {
            "query": np.ascontiguousarray(arr["query"][b, h * half : (h + 1) * half]),
            "key": arr["key"][b],
            "value": arr["value"][b],
        }
        for w in ("Wq", "Wk", "Wv", "Wo", "bq", "bk", "bv", "bo"):
            m[w] = arr[w]
        in_maps.append(m)
    res = run_bass_kernel_spmd(nc, in_maps, list(range(N_CORES)), trace=trace)
    out = np.empty((B, T_full, D), np.float32)
    for core in range(N_CORES):
        b, h = divmod(core, 2)
        out[b, h * half : (h + 1) * half] = res.results[core]["out"]
    return out, res


def kernel(**inputs):
    out, _ = run(inputs, trace=False)
    return out
